# revision 1
# baseline (speedup 1.0000x reference)
"""GPT causal attention (B=2,S=4096,D=768,H=12,HD=64) on 8 NeuronCores.

Sharding: core c handles batch b=c//4 and head-group hg=c%4 (3 heads each).
Per-core kernel (all matmuls bf16, fp32 PSUM accumulate):
  - QKV projections from x^T (features-on-partition layout)
  - scores computed TRANSPOSED: S^T[tk, tq] = K Q^T so softmax-exp output
    feeds P@V directly as the moving operand (no transposes)
  - softmax denominators via a ones-column appended to V (row 64 of ctx psum)
  - exact causal masking: per key-chunk column trimming + triangular mask
    on the diagonal 128x128 block
  - normalization folded in before the output projection; output written
    transposed (outT [768, 4096]); host sums the 4 head-group partials.
"""
import sys

sys.path.insert(0, "/opt/trn_rl_repo")

import numpy as np
import ml_dtypes

import concourse.bass as bass
import concourse.tile as tile
from concourse import bacc, mybir
from concourse.bass_utils import run_bass_kernel_spmd

B, S, D, H, HD = 2, 4096, 768, 12, 64
N_CORES = 8
HPC = 3           # heads per core
DH = HPC * HD     # 192 per-core qkv dims
KD = D // 128     # 6 contraction chunks
QSB = 1024        # query superblock width
NQSB = S // QSB   # 4
NKC = S // 128    # 32 key chunks
NTC = S // 512    # 8 512-wide token chunks

f32 = mybir.dt.float32
bf16 = mybir.dt.bfloat16
BF = ml_dtypes.bfloat16
EXP = mybir.ActivationFunctionType.Exp

_CACHE = {}


def _ranges(off):
    # psum-bank-safe column ranges [off, 1024)
    if off < 512:
        return [(off, 512), (512, 1024)]
    return [(off, 1024)]


def build(repeat=1):
    nc = bacc.Bacc("TRN2", target_bir_lowering=False, debug=False,
                   num_devices=N_CORES)
    xT = nc.dram_tensor("xT", [D, S], bf16, kind="ExternalInput").ap()
    wqT = nc.dram_tensor("wqT", [D, DH], bf16, kind="ExternalInput").ap()
    wkT = nc.dram_tensor("wkT", [D, DH], bf16, kind="ExternalInput").ap()
    wvT = nc.dram_tensor("wvT", [D, DH], bf16, kind="ExternalInput").ap()
    woT = nc.dram_tensor("woT", [DH, D], bf16, kind="ExternalInput").ap()
    bqv = nc.dram_tensor("bqv", [1, DH], bf16, kind="ExternalInput").ap()
    bkv = nc.dram_tensor("bkv", [1, DH], bf16, kind="ExternalInput").ap()
    bvv = nc.dram_tensor("bvv", [1, DH], bf16, kind="ExternalInput").ap()
    tmk = nc.dram_tensor("tmk", [128, 128], bf16, kind="ExternalInput").ap()
    outT = nc.dram_tensor("outT", [D, S], f32, kind="ExternalOutput").ap()
    den_d = nc.dram_tensor("den_d", [HPC, S], f32).ap()
    rcp_d = nc.dram_tensor("rcp_d", [HPC, S], f32).ap()

    with tile.TileContext(nc) as tc:
        import contextlib
        with contextlib.ExitStack() as ctx:
            sb = ctx.enter_context(tc.tile_pool(name="sb", bufs=1))
            # resident inputs
            xt = []
            for k in range(KD):
                t = sb.tile([128, S], bf16, tag=f"xt{k}", name=f"xt{k}")
                nc.sync.dma_start(out=t, in_=xT[k * 128:(k + 1) * 128, :])
                xt.append(t)
            wq_sb = sb.tile([128, KD, DH], bf16, tag="wq", name="wq_sb")
            wk_sb = sb.tile([128, KD, DH], bf16, tag="wk", name="wk_sb")
            wv_sb = sb.tile([128, KD, DH], bf16, tag="wv", name="wv_sb")
            for t, src in ((wq_sb, wqT), (wk_sb, wkT), (wv_sb, wvT)):
                nc.sync.dma_start(
                    out=t, in_=src.rearrange("(ko p) m -> p ko m", p=128))
            wo_a = sb.tile([128, D], bf16, tag="woa", name="wo_a")
            wo_b = sb.tile([64, D], bf16, tag="wob", name="wo_b")
            nc.sync.dma_start(out=wo_a, in_=woT[0:128, :])
            nc.sync.dma_start(out=wo_b, in_=woT[128:DH, :])
            bq_sb = sb.tile([1, DH], bf16, tag="bq", name="bq_sb")
            bk_sb = sb.tile([1, DH], bf16, tag="bk", name="bk_sb")
            bv_sb = sb.tile([1, DH], bf16, tag="bv", name="bv_sb")
            for t, src in ((bq_sb, bqv), (bk_sb, bkv), (bv_sb, bvv)):
                nc.sync.dma_start(out=t, in_=src)
            mask_sb = sb.tile([128, 128], bf16, tag="mk", name="mask_sb")
            nc.sync.dma_start(out=mask_sb, in_=tmk)
            ones512 = sb.tile([1, 512], bf16, tag="o5", name="ones512")
            nc.vector.memset(ones512, 1.0)
            ones128 = sb.tile([1, 128], bf16, tag="o1", name="ones128")
            nc.vector.memset(ones128, 1.0)

            # qkv storage
            QT_a = sb.tile([128, S], bf16, tag="qta", name="QT_a")
            QT_b = sb.tile([64, S], bf16, tag="qtb", name="QT_b")
            KT_a = sb.tile([128, S], bf16, tag="kta", name="KT_a")
            KT_b = sb.tile([64, S], bf16, tag="ktb", name="KT_b")
            V_sb = sb.tile([128, NKC, HPC, HD + 1], bf16, tag="vsb",
                           name="V_sb")
            nc.vector.memset(V_sb[:, :, :, HD:HD + 1], 1.0)

            # ---- projections ----
            with tc.tile_pool(name="ppp", bufs=2, space="PSUM") as ppp:
                for wsb, bsb, oa, ob in ((wq_sb, bq_sb, QT_a, QT_b),
                                         (wk_sb, bk_sb, KT_a, KT_b)):
                    for n in range(NTC):
                        cols = slice(n * 512, n * 512 + 512)
                        ps = ppp.tile([128, 512], f32, tag="pp", name="pp")
                        for k in range(KD):
                            nc.tensor.matmul(ps, wsb[:, k, 0:128],
                                             xt[k][:, cols],
                                             start=(k == 0), stop=False)
                        nc.tensor.matmul(ps, bsb[:, 0:128], ones512,
                                         start=False, stop=True)
                        nc.vector.tensor_copy(oa[:, cols], ps)
                        ps2 = ppp.tile([64, 512], f32, tag="pp2", name="pp2")
                        for k in range(KD):
                            nc.tensor.matmul(ps2, wsb[:, k, 128:DH],
                                             xt[k][:, cols],
                                             start=(k == 0), stop=False)
                        nc.tensor.matmul(ps2, bsb[:, 128:DH], ones512,
                                         start=False, stop=True)
                        nc.vector.tensor_copy(ob[:, cols], ps2)
                for t in range(NKC):
                    tcols = slice(t * 128, t * 128 + 128)
                    ps = ppp.tile([128, DH], f32, tag="pv", name="pv")
                    for k in range(KD):
                        nc.tensor.matmul(ps, xt[k][:, tcols], wv_sb[:, k, :],
                                         start=(k == 0), stop=False)
                    nc.tensor.matmul(ps, ones128, bv_sb, start=False,
                                     stop=True)
                    nc.vector.tensor_copy(
                        V_sb[:, t, :, 0:HD],
                        ps.rearrange("p (h d) -> p h d", h=HPC))

            # ---- attention + output projection ----
            with tc.tile_pool(name="sps", bufs=2, space="PSUM") as sps, \
                 tc.tile_pool(name="cps", bufs=1, space="PSUM") as cps, \
                 tc.tile_pool(name="ops", bufs=2, space="PSUM") as ops, \
                 tc.tile_pool(name="att", bufs=3) as att, \
                 tc.tile_pool(name="nrm", bufs=2) as nrm:
                for _rep in range(repeat):
                  for qsb in range(NQSB):
                    q0 = qsb * QSB
                    cn = []
                    for h in range(HPC):
                        if h < 2:
                            kt, qt, pb = KT_a, QT_a, 64 * h
                        else:
                            kt, qt, pb = KT_b, QT_b, 0
                        ctxp = cps.tile([65, QSB], f32, tag="ctx", name="ctx")
                        nkc = 8 * qsb + 8
                        for kc in range(nkc):
                            off = max(0, kc - 8 * qsb) * 128
                            sp = sps.tile([128, QSB], f32, tag="sp", name="sp")
                            for c0, c1 in _ranges(off):
                                nc.tensor.matmul(
                                    sp[:, c0:c1],
                                    kt[pb:pb + 64, kc * 128:kc * 128 + 128],
                                    qt[pb:pb + 64, q0 + c0:q0 + c1],
                                    start=True, stop=True)
                            es = att.tile([128, QSB], bf16, tag="es",
                                          bufs=4, name="es")
                            nc.scalar.activation(es[:, off:QSB],
                                                 sp[:, off:QSB], EXP,
                                                 scale=0.125)
                            if kc >= 8 * qsb:
                                nc.vector.tensor_mul(es[:, off:off + 128],
                                                     es[:, off:off + 128],
                                                     mask_sb)
                            for c0, c1 in _ranges(off):
                                nc.tensor.matmul(
                                    ctxp[:, c0:c1], V_sb[:, kc, h, :],
                                    es[:, c0:c1], start=(kc == 0),
                                    stop=(kc == nkc - 1))
                        # drain ctx (+denominator row 64) to SBUF, base 0
                        cu = nrm.tile([65, QSB], f32, tag="cu", bufs=4,
                                      name="cu")
                        nc.vector.tensor_copy(cu, ctxp)
                        # denominator -> dram -> reciprocal -> dram -> bcast
                        nc.sync.dma_start(out=den_d[h:h + 1, q0:q0 + QSB],
                                          in_=cu[64:65, :])
                        den = nrm.tile([1, QSB], f32, tag="den", name="den")
                        nc.sync.dma_start(out=den,
                                          in_=den_d[h:h + 1, q0:q0 + QSB])
                        rcp = nrm.tile([1, QSB], f32, tag="rcp", name="rcp")
                        nc.vector.reciprocal(rcp, den)
                        nc.sync.dma_start(out=rcp_d[h:h + 1, q0:q0 + QSB],
                                          in_=rcp)
                        rsl = rcp_d[h:h + 1, q0:q0 + QSB]
                        rb = bass.AP(tensor=rsl.tensor, offset=rsl.offset,
                                     ap=[[0, 64], list(rsl.ap[-1])])
                        bc = nrm.tile([64, QSB], f32, tag="bc", bufs=4,
                                      name="bc")
                        nc.sync.dma_start(out=bc, in_=rb)
                        cnh = nrm.tile([64, QSB], bf16, tag="cn", bufs=4,
                                       name="cnh")
                        nc.vector.tensor_mul(cnh, cu[0:64, :], bc)
                        cn.append(cnh)
                    cna = nrm.tile([128, QSB], bf16, tag="cna", name="cna")
                    nc.sync.dma_start(out=cna[0:64, :], in_=cn[0])
                    nc.sync.dma_start(out=cna[64:128, :], in_=cn[1])
                    cnb = cn[2]
                    for tb in range(2):
                        t0 = tb * 512
                        for oc in range(6):
                            po = ops.tile([128, 512], f32, tag="po",
                                          name="po")
                            nc.tensor.matmul(po,
                                             wo_a[:, oc * 128:oc * 128 + 128],
                                             cna[:, t0:t0 + 512],
                                             start=True, stop=False)
                            nc.tensor.matmul(po,
                                             wo_b[:, oc * 128:oc * 128 + 128],
                                             cnb[:, t0:t0 + 512],
                                             start=False, stop=True)
                            osb = nrm.tile([128, 512], f32, tag="osb",
                                           bufs=3, name="osb")
                            nc.vector.tensor_copy(osb, po)
                            nc.sync.dma_start(
                                out=outT[oc * 128:oc * 128 + 128,
                                         q0 + t0:q0 + t0 + 512],
                                in_=osb)
    nc.compile()
    return nc


def _in_maps(x, Wq, bq, Wk, bk, Wv, bv, Wo, bo):
    tri = np.triu(np.ones((128, 128), np.float32)).astype(BF)
    maps = []
    for c in range(N_CORES):
        b, hg = c // 4, c % 4
        sl = slice(DH * hg, DH * hg + DH)
        maps.append({
            "xT": np.ascontiguousarray(x[b].T).astype(BF),
            "wqT": np.ascontiguousarray(Wq[sl, :].T).astype(BF),
            "wkT": np.ascontiguousarray(Wk[sl, :].T).astype(BF),
            "wvT": np.ascontiguousarray(Wv[sl, :].T).astype(BF),
            "woT": np.ascontiguousarray(Wo[:, sl].T).astype(BF),
            "bqv": bq[sl].reshape(1, DH).astype(BF),
            "bkv": bk[sl].reshape(1, DH).astype(BF),
            "bvv": bv[sl].reshape(1, DH).astype(BF),
            "tmk": tri,
        })
    return maps


def kernel(x, Wq, bq, Wk, bk, Wv, bv, Wo, bo):
    if "nc" not in _CACHE:
        _CACHE["nc"] = build()
    nc = _CACHE["nc"]
    maps = _in_maps(x, Wq, bq, Wk, bk, Wv, bv, Wo, bo)
    res = run_bass_kernel_spmd(nc, maps, list(range(N_CORES))).results
    out = np.zeros((B, S, D), np.float32)
    for c in range(N_CORES):
        out[c // 4] += res[c]["outT"].T
    out += bo.astype(np.float32)
    return out

